# revision 1
# baseline (speedup 1.0000x reference)
"""Trainium2 Bass kernel: CLUTRR-style GNN message passing (nn_CLUTRRV4).

Data-parallel across 8 NeuronCores (256 samples/core). Per core, samples are
packed 4-per-group (4 x 32 entity slots = 128 partitions). Entity states stay
SBUF-resident for all 8 message-passing steps; gather/scatter are expressed as
one-hot matmuls with the one-hot matrices generated on-chip from int16 index
tiles via is_equal. Matmuls run in float16 (full-rate PE + fast weight load, ~5e-4 matmul
accuracy); the state S is kept in fp32 master form with an fp16 shadow copy.
N=128 matmuls are widened to N=256 via step-0 duplicated rhs APs to stay in
the f32r full-rate regime.
"""
import sys
import numpy as np

if "/opt/trn_rl_repo" not in sys.path:
    sys.path.append("/opt/trn_rl_repo")

N_ENT, N_REL, D, E = 32, 20, 128, 64
N_STEPS = 8
N_CORES = 8
P = 128
GRP = 4  # samples per group


def _patch_ldw_opt():
    import os
    if os.environ.get("BASS_LDW_OPT") != "1":
        return
    from concourse import bass_utils as bu
    if getattr(bu, "_ldw_opt_patched", False):
        return
    orig = bu.run_command

    def run_command_ldw(cmd, *a, **kw):
        if isinstance(cmd, list):
            cmd = [c.replace("--enable-ldw-opt=false", "--enable-ldw-opt=true")
                   if isinstance(c, str) else c for c in cmd]
        return orig(cmd, *a, **kw)

    bu.run_command = run_command_ldw
    bu._ldw_opt_patched = True


def _build_nc(b_core, n_steps, use_gelu=True):
    from concourse import bacc, mybir
    from concourse.tile import TileContext
    from concourse.masks import make_identity

    f32 = mybir.dt.float32
    f32r = mybir.dt.float16
    i16 = mybir.dt.int16
    AF = mybir.ActivationFunctionType
    OP = mybir.AluOpType
    act_fn = AF.Gelu if use_gelu else AF.Identity

    G = b_core // GRP
    NPAIR = G // 2
    assert G % 4 == 0, "group count must be a multiple of 4 for rel/indeg packing"

    nc = bacc.Bacc()

    def din(name, shape, dtype=f32):
        return nc.declare_dram_parameter(name, list(shape), dtype, isOutput=False)

    d_s0 = din("s0", (P, G * P))
    d_gs = din("gsrc", (G, P, 256), i16)
    d_gt = din("gtgt", (G, P, 256), i16)
    d_gtc = din("gtc", (P, 2 * G), i16)
    d_rel = din("reloh", (G // 4, P, 256), f32r)
    d_rt4 = din("reltab4", (P, 256), f32r)
    d_ind = din("indeg", (G // 4, P, P), f32r)
    d_b2r = din("b2row", (P, P), f32r)
    d_qoh = din("qoh", (G, P, 8))
    d_w1ac = din("w1ac", (P, 512), f32r)
    d_w2m = din("w2m", (P, 256), f32r)
    d_w1u = din("w1u", (P, 512), f32r)
    d_w2u = din("w2u", (P, 256), f32r)
    d_b1u = din("b1u", (P, 2))
    d_b2u = din("b2u", (P, 1))
    d_cw1 = din("cw1", (P, 256))
    d_cb1 = din("cb1", (P, 1))
    d_cw2 = din("cw2", (P, 20))
    d_cb2 = din("cb2", (20, 1))
    d_out = nc.declare_dram_parameter("out", [20, b_core], f32, isOutput=True)

    with TileContext(nc) as tc:
        with (
            tc.tile_pool(name="c", bufs=1) as cp,
            tc.tile_pool(name="w", bufs=4) as wp,
            tc.tile_pool(name="pA", bufs=2, space="PSUM") as pA,
            tc.tile_pool(name="pH1", bufs=2, space="PSUM") as pH1,
            tc.tile_pool(name="pM", bufs=1, space="PSUM") as pM,
            tc.tile_pool(name="pG", bufs=1, space="PSUM") as pG,
            tc.tile_pool(name="pH3", bufs=1, space="PSUM") as pH3,
            tc.tile_pool(name="pS", bufs=1, space="PSUM") as pS,
        ):
            def cload(name, shape, dram, dtype=f32):
                t = cp.tile(list(shape), dtype, tag=name)
                nc.sync.dma_start(t[:], dram[:])
                return t

            w1ac = cload("w1ac", (P, 512), d_w1ac, f32r)
            w2m = cload("w2m", (P, 256), d_w2m, f32r)
            w1u = cload("w1u", (P, 512), d_w1u, f32r)
            w2u = cload("w2u", (P, 256), d_w2u, f32r)
            rt4 = cload("rt4", (P, 256), d_rt4, f32r)
            b2r = cload("b2r", (P, P), d_b2r, f32r)
            b1u = cload("b1u", (P, 2), d_b1u)
            b2u = cload("b2u", (P, 1), d_b2u)
            cw1 = cload("cw1", (P, 256), d_cw1)
            cb1 = cload("cb1", (P, 1), d_cb1)
            cw2 = cload("cw2", (P, 20), d_cw2)
            cb2 = cload("cb2", (20, 1), d_cb2)
            gtc = cload("gtc", (P, 2 * G), d_gtc, i16)

            eiota = cp.tile([P, 1], i16, tag="eiota")
            nc.gpsimd.iota(eiota[:], pattern=[[0, 1]], base=0, channel_multiplier=1)
            fiota = cp.tile([P, P], i16, tag="fiota")
            nc.gpsimd.iota(fiota[:], pattern=[[1, P]], base=0, channel_multiplier=0)
            ident = cp.tile([P, P], f32, tag="ident")
            make_identity(nc, ident[:])
            outsb = cp.tile([20, b_core], f32, tag="outsb")

            S, SR = [], []
            GS, GT, QOH = [None] * G, [None] * G, [None] * G
            RELP, INDP = [None] * (G // 4), [None] * (G // 4)
            for p in range(NPAIR):
                if p % 2 == 0:
                    j = p // 2
                    t = cp.tile([P, 256], f32r, tag=f"rp{j}")
                    nc.sync.dma_start(t[:], d_rel[j])
                    RELP[j] = t
                    t = cp.tile([P, P], f32r, tag=f"ip{j}")
                    nc.sync.dma_start(t[:], d_ind[j])
                    INDP[j] = t
                t = cp.tile([P, 256], f32, tag=f"S{p}")
                nc.sync.dma_start(t[:], d_s0[:, p * 256:(p + 1) * 256])
                S.append(t)
                t2 = cp.tile([P, 256], f32r, tag=f"Sr{p}")
                nc.gpsimd.tensor_copy(t2[:], t[:])
                SR.append(t2)
                for g in (2 * p, 2 * p + 1):
                    t = cp.tile([P, 256], i16, tag=f"gs{g}")
                    nc.sync.dma_start(t[:], d_gs[g])
                    GS[g] = t
                    t = cp.tile([P, 256], i16, tag=f"gt{g}")
                    nc.sync.dma_start(t[:], d_gt[g])
                    GT[g] = t
                    t = cp.tile([P, 8], f32, tag=f"q{g}")
                    nc.sync.dma_start(t[:], d_qoh[g])
                    QOH[g] = t

            mm = nc.tensor.matmul

            def dup2(ap_):
                """(K, n) AP -> (K, 2, n) with step-0 middle dim (rhs widening)."""
                k, n = ap_.shape
                return ap_[:, None, :].to_broadcast([k, 2, n])

            for t_step in range(n_steps):
                for p in range(NPAIR):
                    agg = pG.tile([P, 256], f32, tag="agg")
                    for gi in range(2):
                        g = 2 * p + gi
                        rb = (g % 4) * 32
                        # A = [S@W1a | S@W1c] in natural (slot-major) layout
                        aps = pA.tile([P, 512], f32, tag="aps")
                        mm(aps[:], lhsT=SR[p][:, gi * P:(gi + 1) * P], rhs=w1ac[:],
                           start=True, stop=True)
                        asb = wp.tile([P, 512], f32r, tag="asb")
                        nc.vector.tensor_copy(asb[:], aps[:])
                        # ent-major one-hots (DVE)
                        ohs = wp.tile([P, 256], f32r, tag="ohs")
                        nc.vector.tensor_tensor(
                            ohs[:], GS[g][:], eiota[:].to_broadcast([P, 256]),
                            op=OP.is_equal)
                        oht = wp.tile([P, 256], f32r, tag="oht")
                        nc.vector.tensor_tensor(
                            oht[:], GT[g][:], eiota[:].to_broadcast([P, 256]),
                            op=OP.is_equal)
                        # h1 = rel_bias + onehot_src@A + onehot_tgt@Bt (per featchunk)
                        h1 = pH1.tile([P, 512], f32, tag="h1")
                        for F in range(2):
                            o = h1[:, F * 256:(F + 1) * 256]
                            mm(o, lhsT=rt4[rb:rb + 20, F * P:(F + 1) * P],
                               rhs=RELP[g // 4][rb:rb + 20, :], start=True, stop=False,
                               tile_position=(rb, 0))
                            mm(o, lhsT=asb[:, F * P:(F + 1) * P], rhs=ohs[:],
                               start=False, stop=False)
                            mm(o, lhsT=asb[:, 256 + F * P:256 + (F + 1) * P],
                               rhs=oht[:], start=False, stop=True)
                        h1g = wp.tile([P, 512], f32r, tag="h1g")
                        nc.scalar.activation(h1g[:], h1[:], act_fn)
                        # msg layer 2, emitted edge-major
                        msg = pM.tile([P, 256], f32, tag="msg")
                        for ec in range(2):
                            o = msg[:, ec * P:(ec + 1) * P]
                            for F in range(2):
                                mm(o, lhsT=h1g[:, F * 256 + ec * P:F * 256 + (ec + 1) * P],
                                   rhs=w2m[:, F * P:(F + 1) * P],
                                   start=(F == 0), stop=(F == 1))
                        msb = wp.tile([P, 256], f32r, tag="msb")
                        nc.scalar.copy(msb[:], msg[:])
                        # edge-major masked tgt one-hot (DVE)
                        ohe = wp.tile([P, 256], f32r, tag="ohe")
                        for ec in range(2):
                            col = gtc[:, g * 2 + ec:g * 2 + ec + 1]
                            nc.vector.tensor_tensor(
                                ohe[:, ec * P:(ec + 1) * P],
                                col.to_broadcast([P, P]), fiota[:], op=OP.is_equal)
                        # scatter-add + msg_b2*indegree fold
                        o = agg[:, gi * P:(gi + 1) * P]
                        mm(o, lhsT=msb[:, 0:P], rhs=ohe[:, 0:P],
                           start=True, stop=False)
                        mm(o, lhsT=msb[:, P:256], rhs=ohe[:, P:256],
                           start=False, stop=False)
                        mm(o, lhsT=b2r[rb:rb + 1, :],
                           rhs=INDP[g // 4][rb:rb + 1, :],
                           start=False, stop=True, tile_position=(rb, 0))
                    # update MLP over the pair (256 slot cols)
                    gsb = wp.tile([P, 256], f32r, tag="gsb")
                    nc.scalar.copy(gsb[:], agg[:])
                    h3 = pH3.tile([P, 512], f32, tag="h3")
                    for mc in range(2):
                        o = h3[:, mc * 256:(mc + 1) * 256]
                        mm(o, lhsT=w1u[:, mc * P:(mc + 1) * P], rhs=SR[p][:],
                           start=True, stop=False)
                        mm(o, lhsT=w1u[:, 256 + mc * P:256 + (mc + 1) * P], rhs=gsb[:],
                           start=False, stop=True)
                    h3g = wp.tile([P, 512], f32r, tag="h3g")
                    for mc in range(2):
                        nc.scalar.activation(
                            h3g[:, mc * 256:(mc + 1) * 256],
                            h3[:, mc * 256:(mc + 1) * 256], act_fn,
                            bias=b1u[:, mc:mc + 1])
                    sn = pS.tile([P, 256], f32, tag="sn")
                    for kc in range(2):
                        mm(sn[:], lhsT=w2u[:, kc * P:(kc + 1) * P],
                           rhs=h3g[:, kc * 256:(kc + 1) * 256],
                           start=(kc == 0), stop=(kc == 1))
                    # S += sn + b2u (fp32 master), then refresh the f32r shadow
                    nc.vector.scalar_tensor_tensor(
                        out=S[p][:], in0=sn[:], scalar=b2u[:, 0:1], in1=S[p][:],
                        op0=OP.add, op1=OP.add)
                    nc.gpsimd.tensor_copy(SR[p][:], S[p][:])

            # classifier head (fp32 throughout; tiny)
            nbatch = (G + 15) // 16
            for bq in range(nbatch):
                jn = min(16, G - bq * 16)
                qps = pH1.tile([P, P], f32, tag="h1")
                for j in range(jn):
                    g = bq * 16 + j
                    p2, gi = divmod(g, 2)
                    stp = pA.tile([P, P], f32, tag="aps")
                    nc.tensor.transpose(stp[:], S[p2][:, gi * P:(gi + 1) * P], ident[:])
                    sts = wp.tile([P, P], f32, tag="sts")
                    nc.vector.tensor_copy(sts[:], stp[:])
                    mm(qps[:, j * 8:(j + 1) * 8], lhsT=sts[:], rhs=QOH[g][:],
                       start=True, stop=True)
                qcat = wp.tile([P, P], f32, tag="qcat")
                nc.vector.tensor_copy(qcat[:, 0:jn * 8], qps[:, 0:jn * 8])
                qv = qcat[:, 0:jn * 8].rearrange("p (g t f) -> p g t f", t=2, f=4)
                ncols = jn * 4
                hps = pM.tile([P, ncols], f32, tag="msg")
                mm(hps[:], lhsT=cw1[:, 0:P], rhs=qv[:, :, 0, :], start=True, stop=False)
                mm(hps[:], lhsT=cw1[:, P:256], rhs=qv[:, :, 1, :], start=False, stop=True)
                hg = wp.tile([P, ncols], f32, tag="hg")
                nc.scalar.activation(hg[:], hps[:], act_fn, bias=cb1[:, 0:1])
                ops_ = pG.tile([20, ncols], f32, tag="agg")
                mm(ops_[:], lhsT=cw2[:], rhs=hg[:], start=True, stop=True)
                nc.scalar.activation(
                    outsb[:, bq * 64:bq * 64 + ncols], ops_[:], AF.Identity,
                    bias=cb2[:, 0:1])
            nc.sync.dma_start(d_out[:], outsb[:])

    nc.finalize()
    return nc


def _host_prep_shared(inp, b_core):
    f = np.float32
    ee = np.asarray(inp["entity_embed"], f)
    w1 = np.asarray(inp["msg_W1"], f)
    reltab = np.asarray(inp["rel_embed"], f) @ w1[128:256] + np.asarray(inp["msg_b1"], f)
    rt4 = np.zeros((P, 256), f)
    b2r = np.zeros((P, P), f)
    for b in range(4):
        rt4[b * 32:b * 32 + 20] = reltab
        b2r[b * 32] = np.asarray(inp["msg_b2"], f)
    w2m_ = np.asarray(inp["msg_W2"], f)
    w1u_ = np.asarray(inp["upd_W1"], f)
    w2u_ = np.asarray(inp["upd_W2"], f)
    cw1_ = np.asarray(inp["cls_W1"], f)
    h = np.float16
    return {
        "s0": np.tile(ee.T, (1, b_core)).astype(f),
        "reltab4": rt4.astype(h),
        "b2row": b2r.astype(h),
        "w1ac": np.concatenate([w1[0:128], w1[256:384]], axis=1).astype(h),
        "w2m": np.concatenate([w2m_[0:128], w2m_[128:256]], axis=1).astype(h),
        "w1u": np.concatenate(
            [w1u_[0:128, 0:128], w1u_[0:128, 128:256],
             w1u_[128:256, 0:128], w1u_[128:256, 128:256]], axis=1).astype(h),
        "w2u": np.concatenate([w2u_[0:128], w2u_[128:256]], axis=1).astype(h),
        "b1u": np.asarray(inp["upd_b1"], f).reshape(2, 128).T.copy(),
        "b2u": np.asarray(inp["upd_b2"], f).reshape(128, 1).copy(),
        "cw1": np.concatenate([cw1_[0:128], cw1_[128:256]], axis=1).astype(f),
        "cb1": np.asarray(inp["cls_b1"], f).reshape(128, 1).copy(),
        "cw2": np.asarray(inp["cls_W2"], f).copy(),
        "cb2": np.asarray(inp["cls_b2"], f).reshape(20, 1).copy(),
    }


def _host_prep_core(inp, c, b_core):
    f = np.float32
    sl = slice(c * b_core, (c + 1) * b_core)
    src = np.asarray(inp["edge_src"])[sl].astype(np.int64)
    tgt = np.asarray(inp["edge_tgt"])[sl].astype(np.int64)
    rel = np.asarray(inp["edge_rel"])[sl].astype(np.int64)
    ne = np.asarray(inp["n_edges"])[sl].astype(np.int64)
    qs = np.asarray(inp["query_src"])[sl].astype(np.int64)
    qt = np.asarray(inp["query_tgt"])[sl].astype(np.int64)
    G = b_core // GRP

    mask = (np.arange(E)[None, :] < ne[:, None])
    soff = (np.arange(b_core) % GRP)[:, None] * 32
    gs = np.where(mask, soff + src, 255).astype(np.int16).reshape(G, 256)
    gt = np.where(mask, soff + tgt, 255).astype(np.int16).reshape(G, 256)

    relg = rel.reshape(G, 256)
    reloh = np.zeros((G // 4, P, 256), f)
    oh = (relg[:, None, :] == np.arange(20)[None, :, None]).astype(f)
    reloh.reshape(G // 4, 4, 32, 256)[:, :, :20] = oh.reshape(G // 4, 4, 20, 256)

    ind = np.zeros((b_core, 32), f)
    np.add.at(ind, (np.repeat(np.arange(b_core), E), tgt.ravel()),
              mask.ravel().astype(f))
    indp = np.zeros((G // 4, P, P), f)
    indp.reshape(G // 4, 4, 32, P)[:, :, 0, :] = ind.reshape(G // 4, 4, P)

    qoh = np.zeros((G, P, 8), f)
    s_all = np.arange(b_core)
    gidx = s_all // GRP
    sg = s_all % GRP
    qoh[gidx, sg * 32 + qs, sg] = 1.0
    qoh[gidx, sg * 32 + qt, 4 + sg] = 1.0

    return {
        "gsrc": np.ascontiguousarray(np.broadcast_to(gs[:, None, :], (G, P, 256))),
        "gtgt": np.ascontiguousarray(np.broadcast_to(gt[:, None, :], (G, P, 256))),
        "gtc": np.ascontiguousarray(gt.reshape(2 * G, P).T),
        "reloh": reloh.astype(np.float16),
        "indeg": indp.astype(np.float16),
        "qoh": qoh,
    }


_CACHE = {}


def kernel(**inputs):
    b = np.asarray(inputs["edge_src"]).shape[0]
    b_core = b // N_CORES
    _patch_ldw_opt()
    key = b_core
    if key not in _CACHE:
        _CACHE[key] = _build_nc(b_core, N_STEPS, use_gelu=True)
    nc = _CACHE[key]

    shared = _host_prep_shared(inputs, b_core)
    in_maps = []
    for c in range(N_CORES):
        m = dict(shared)
        m.update(_host_prep_core(inputs, c, b_core))
        in_maps.append(m)

    from concourse.bass_utils import run_bass_kernel_spmd
    res = run_bass_kernel_spmd(nc, in_maps, core_ids=list(range(N_CORES)))
    out = np.concatenate([r["out"].T for r in res.results], axis=0)
    return np.ascontiguousarray(out, dtype=np.float32)



# revision 9
# speedup vs baseline: 1.3127x; 1.3127x over previous
"""Trainium2 Bass kernel: CLUTRR-style GNN message passing (nn_CLUTRRV4).

Data-parallel across 8 NeuronCores. Samples are packed 4-per-group
(4 x 32 entity slots = 128 partitions); sample->group assignment is an
LPT bin-packing so that each group's VALID edges fit in EC=128 packed
edge columns (vs 256 naive), skipping all masked-edge compute.

All one-hot gather/scatter/rel matrices are precomputed on the host and
DMA'd once (they are step-invariant); nothing is generated on-chip.
Entity state S is fp16-only (tolerance 2e-2 >> fp16 error here).

Per step, per group: S is transposed (PE) to slot-major, src/tgt states
are gathered via one-hot matmuls, the message MLP layer 1 uses fixed
weight blocks (rel contribution via the 20-row band trick), messages are
scattered back with the edge-major one-hot, and the update MLP runs per
quad (4 groups) with N=512 matmuls. Emission is a software-pipelined
flat loop (modulo schedule) so the PE never waits on the DVE/Act
converts; PSUM is budgeted at exactly 8 banks.
"""
import sys
import numpy as np

if "/opt/trn_rl_repo" not in sys.path:
    sys.path.append("/opt/trn_rl_repo")

N_ENT, N_REL, D, E = 32, 20, 128, 64
N_STEPS = 8
N_CORES = 8
P = 128
EC = 128          # packed edge columns per group
GRP = 4           # samples per group


def _build_nc(G, n_steps):
    from concourse import bacc, mybir
    from concourse.tile import TileContext
    from concourse.masks import make_identity

    f32 = mybir.dt.float32
    f16 = mybir.dt.float16
    AF = mybir.ActivationFunctionType
    OP = mybir.AluOpType

    assert G % 4 == 0
    NQ = G // 4
    SLOTS = G * P

    nc = bacc.Bacc()

    def din(name, shape, dtype=f32):
        return nc.declare_dram_parameter(name, list(shape), dtype, isOutput=False)

    d_s0 = din("s0", (P, SLOTS), f16)
    d_oh = din("oh", (G, P, 3 * EC), f16)
    d_relt = din("relt", (NQ, P, EC), f16)
    d_indt = din("indt", (NQ, P, EC), f16)
    d_qoh = din("qoh", (P, G * 8), f16)
    d_w1s = din("w1s", (P, 256), f16)
    d_w1t = din("w1t", (P, 256), f16)
    d_rt4 = din("rt4", (P, 256), f16)
    d_w2m = din("w2m", (P, 256), f16)
    d_w1u = din("w1u", (P, 512), f16)
    d_w2u = din("w2u", (P, 256), f16)
    d_b2row = din("b2row", (P, P), f16)
    d_b1u = din("b1u", (P, 2))
    d_b2u = din("b2u", (P, 1))
    d_cw1 = din("cw1", (P, 256))
    d_cb1 = din("cb1", (P, 1))
    d_cw2 = din("cw2", (P, 20))
    d_cb2 = din("cb2", (20, 1))
    d_out = nc.declare_dram_parameter("out", [20, G * GRP], f32, isOutput=True)

    with TileContext(nc) as tc:
        with (
            tc.tile_pool(name="c", bufs=1) as cp,
            tc.tile_pool(name="w", bufs=2) as wp,
            tc.tile_pool(name="pST", bufs=2, space="PSUM") as pST,
            tc.tile_pool(name="pGA", bufs=1, space="PSUM") as pGA,
            tc.tile_pool(name="pH1", bufs=1, space="PSUM") as pH1,
            tc.tile_pool(name="pMS", bufs=1, space="PSUM") as pMS,
            tc.tile_pool(name="pAG", bufs=1, space="PSUM") as pAG,
            tc.tile_pool(name="pUP", bufs=2, space="PSUM") as pUP,
        ):
            def cload(name, shape, dram, dtype=f32):
                t = cp.tile(list(shape), dtype, tag=name, name=name)
                nc.sync.dma_start(t[:], dram[:])
                return t

            w1s = cload("w1s", (P, 256), d_w1s, f16)
            w1t = cload("w1t", (P, 256), d_w1t, f16)
            rt4 = cload("rt4", (P, 256), d_rt4, f16)
            w2m = cload("w2m", (P, 256), d_w2m, f16)
            w1u = cload("w1u", (P, 512), d_w1u, f16)
            w2u = cload("w2u", (P, 256), d_w2u, f16)
            b2row = cload("b2row", (P, P), d_b2row, f16)
            b1u = cload("b1u", (P, 2), d_b1u)
            b2u = cload("b2u", (P, 1), d_b2u)
            cw1 = cload("cw1", (P, 256), d_cw1)
            cb1 = cload("cb1", (P, 1), d_cb1)
            cw2 = cload("cw2", (P, 20), d_cw2)
            cb2 = cload("cb2", (20, 1), d_cb2)
            qoh = cload("qoh", (P, G * 8), d_qoh, f16)

            ident = cp.tile([P, P], f16, tag="ident", name="ident")
            make_identity(nc, ident[:])

            S = cp.tile([P, SLOTS], f16, tag="S", name="S")
            for q in range(NQ):
                nc.sync.dma_start(S[:, q * 512:(q + 1) * 512],
                                  d_s0[:, q * 512:(q + 1) * 512])
            OH = cp.tile([P, G * 3 * EC], f16, tag="OH", name="OH")
            for g in range(G):
                nc.sync.dma_start(OH[:, g * 384:(g + 1) * 384], d_oh[g])
            RELT, INDT = [], []
            for q in range(NQ):
                t = cp.tile([P, EC], f16, tag=f"relt{q}", name=f"relt{q}")
                nc.sync.dma_start(t[:], d_relt[q])
                RELT.append(t)
                t = cp.tile([P, EC], f16, tag=f"indt{q}", name=f"indt{q}")
                nc.sync.dma_start(t[:], d_indt[q])
                INDT.append(t)

            outsb = cp.tile([20, G * GRP], f32, tag="outsb", name="outsb")

            def ohs(g):
                return OH[:, g * 384:g * 384 + EC]

            def oht(g):
                return OH[:, g * 384 + EC:g * 384 + 2 * EC]

            def ohe(g):
                return OH[:, g * 384 + 2 * EC:g * 384 + 3 * EC]

            mm = nc.tensor.matmul
            st_ = {}
            sts_t, gtb_t, h1g_t, msb_t, agb_t = {}, {}, {}, {}, {}

            # --- pipeline stages ------------------------------------------
            def st_stage(t, g):
                q, j = divmod(g, 4)
                if j == 0:
                    st_['stp'] = pST.tile([P, 512], f16, tag="st", name="stp")
                nc.tensor.transpose(st_['stp'][:, j * P:(j + 1) * P],
                                    S[:, g * P:(g + 1) * P], ident[:])
                if j == 3:
                    t_ = wp.tile([P, 512], f16, tag="sts", name="sts")
                    nc.vector.tensor_copy(t_[:], st_['stp'][:])
                    sts_t[q] = t_

            def ga_stage(t, g):
                q = g // 4
                gi = g % 2
                if gi == 0:
                    st_['gap'] = pGA.tile([P, 512], f32, tag="ga", name="gap")
                gp = st_['gap']
                sts = sts_t[q]
                j = g % 4
                mm(gp[:, gi * 256:gi * 256 + EC],
                   lhsT=sts[:, j * P:(j + 1) * P], rhs=ohs(g),
                   start=True, stop=True)
                mm(gp[:, gi * 256 + EC:gi * 256 + 2 * EC],
                   lhsT=sts[:, j * P:(j + 1) * P], rhs=oht(g),
                   start=True, stop=True)
                if gi == 1:
                    t_ = wp.tile([P, 512], f16, tag="gtb", name="gtb")
                    if (g // 2) % 2 == 0:
                        nc.scalar.copy(t_[:], gp[:])
                    else:
                        nc.vector.tensor_copy(t_[:], gp[:])
                    gtb_t[g // 2] = t_

            def h1_stage(t, g):
                q = g // 4
                gi = g % 2
                if gi == 0:
                    st_['h1p'] = pH1.tile([P, 512], f32, tag="h1", name="h1p")
                hp = st_['h1p']
                gtb = gtb_t[g // 2]
                goff = gi * 256
                rb = (g % 4) * 32
                for c in (0, 1):
                    o = hp[:, gi * 256 + c * EC:gi * 256 + (c + 1) * EC]
                    mm(o, lhsT=rt4[rb:rb + 20, c * P:(c + 1) * P],
                       rhs=RELT[q][rb:rb + 20, :], start=True, stop=False,
                       tile_position=(rb, 0))
                    mm(o, lhsT=w1s[:, c * P:(c + 1) * P],
                       rhs=gtb[:, goff:goff + EC], start=False, stop=False)
                    mm(o, lhsT=w1t[:, c * P:(c + 1) * P],
                       rhs=gtb[:, goff + EC:goff + 2 * EC],
                       start=False, stop=True)
                if gi == 1:
                    t_ = wp.tile([P, 512], f16, tag="h1g", name="h1g")
                    nc.scalar.activation(t_[:], hp[:], AF.Gelu)
                    h1g_t[g // 2] = t_

            def ms_stage(t, g):
                j = g % 4
                if j == 0:
                    st_['msp'] = pMS.tile([P, 512], f32, tag="ms", name="msp")
                mp = st_['msp']
                h1g = h1g_t[g // 2]
                hoff = (g % 2) * 256
                o = mp[:, j * P:(j + 1) * P]
                for c in (0, 1):
                    mm(o, lhsT=h1g[:, hoff + c * P:hoff + (c + 1) * P],
                       rhs=w2m[:, c * P:(c + 1) * P],
                       start=(c == 0), stop=(c == 1))
                if j == 3:
                    t_ = wp.tile([P, 512], f16, tag="msb", name="msb")
                    nc.vector.tensor_copy(t_[:], mp[:])
                    msb_t[g // 4] = t_

            def sc_stage(t, g):
                q, j = divmod(g, 4)
                if j == 0:
                    st_['agp'] = pAG.tile([P, 512], f32, tag="ag", name="agp")
                ap_ = st_['agp']
                msb = msb_t[q]
                rb = j * 32
                o = ap_[:, j * P:(j + 1) * P]
                mm(o, lhsT=msb[:, j * P:(j + 1) * P], rhs=ohe(g),
                   start=True, stop=False)
                mm(o, lhsT=b2row[rb:rb + 1, :], rhs=INDT[q][rb:rb + 1, :],
                   start=False, stop=True, tile_position=(rb, 0))
                if j == 3:
                    t_ = wp.tile([P, 512], f16, tag="agb", name="agb")
                    nc.vector.tensor_copy(t_[:], ap_[:])
                    agb_t[q] = t_

            def up_stage(t, w):
                q, ph = divmod(w, 4)
                if ph == 0:
                    h3a = pUP.tile([P, 512], f32, tag="up", name="h3a")
                    h3b = pUP.tile([P, 512], f32, tag="up", name="h3b")
                    st_['h3a'], st_['h3b'] = h3a, h3b
                    for mc, hb in ((0, h3a), (1, h3b)):
                        mm(hb[:], lhsT=w1u[:, mc * P:(mc + 1) * P],
                           rhs=S[:, q * 512:(q + 1) * 512],
                           start=True, stop=False)
                        mm(hb[:], lhsT=w1u[:, 256 + mc * P:256 + (mc + 1) * P],
                           rhs=agb_t[q][:], start=False, stop=True)
                elif ph == 1:
                    t_ = wp.tile([P, 1024], f16, tag="h3g", name="h3g")
                    nc.scalar.activation(t_[:, 0:512], st_['h3a'][:], AF.Gelu,
                                         bias=b1u[:, 0:1])
                    nc.scalar.activation(t_[:, 512:1024], st_['h3b'][:],
                                         AF.Gelu, bias=b1u[:, 1:2])
                    st_['h3g'] = t_
                elif ph == 2:
                    sn = pUP.tile([P, 512], f32, tag="up", name="sn")
                    st_['sn'] = sn
                    for kc in (0, 1):
                        mm(sn[:], lhsT=w2u[:, kc * P:(kc + 1) * P],
                           rhs=st_['h3g'][:, kc * 512:(kc + 1) * 512],
                           start=(kc == 0), stop=(kc == 1))
                else:
                    nc.vector.scalar_tensor_tensor(
                        out=S[:, q * 512:(q + 1) * 512], in0=st_['sn'][:],
                        scalar=b2u[:, 0:1], in1=S[:, q * 512:(q + 1) * 512],
                        op0=OP.add, op1=OP.add)

            # --- software-pipelined flat loop -----------------------------
            # Stages are emitted deepest-offset first within each unit so
            # that every read of a rotating tile precedes the alloc of the
            # generation that reuses its buffer (WAR legality).
            assert G >= 28, "modulo-schedule offsets need G >= 28"
            offs = (24, 19, 14, 10, 6, 0)
            stages = (up_stage, sc_stage, ms_stage, h1_stage, ga_stage,
                      st_stage)
            total = n_steps * G + offs[0] + 4
            for U in range(total):
                for off, fn in zip(offs, stages):
                    v = U - off
                    if v < 0:
                        continue
                    t, g = divmod(v, G)
                    if t < n_steps:
                        fn(t, g)

            # --- classifier head ------------------------------------------
            nbatch = (G + 15) // 16
            for bq in range(nbatch):
                jn = min(16, G - bq * 16)
                qps = pGA.tile([P, 128], f32, tag="ga", name="qps")
                for j in range(jn):
                    g = bq * 16 + j
                    jj = j % 4
                    if jj == 0:
                        st_['cstp'] = pST.tile([P, 512], f16, tag="st",
                                               name="cstp")
                    nc.tensor.transpose(st_['cstp'][:, jj * P:(jj + 1) * P],
                                        S[:, g * P:(g + 1) * P], ident[:])
                    if jj == 3 or j == jn - 1:
                        t_ = wp.tile([P, 512], f16, tag="sts", name="csts")
                        nc.vector.tensor_copy(t_[:, 0:(jj + 1) * P],
                                              st_['cstp'][:, 0:(jj + 1) * P])
                        # consume immediately: qps gathers for this sub-batch
                        for j2 in range(j - jj, j + 1):
                            g2 = bq * 16 + j2
                            mm(qps[:, j2 * 8:(j2 + 1) * 8],
                               lhsT=t_[:, (j2 % 4) * P:(j2 % 4 + 1) * P],
                               rhs=qoh[:, g2 * 8:(g2 + 1) * 8],
                               start=True, stop=True)
                qcat = wp.tile([P, 128], f32, tag="qcat", name="qcat")
                nc.vector.tensor_copy(qcat[:, 0:jn * 8], qps[:, 0:jn * 8])
                qv = qcat[:, 0:jn * 8].rearrange("p (g t f) -> p g t f",
                                                 t=2, f=4)
                hps = pMS.tile([P, 64], f32, tag="ms", name="hps")
                mm(hps[:, 0:jn * 4], lhsT=cw1[:, 0:P], rhs=qv[:, :, 0, :],
                   start=True, stop=False)
                mm(hps[:, 0:jn * 4], lhsT=cw1[:, P:256], rhs=qv[:, :, 1, :],
                   start=False, stop=True)
                hg = wp.tile([P, 64], f32, tag="hg", name="hg")
                nc.scalar.activation(hg[:, 0:jn * 4], hps[:, 0:jn * 4],
                                     AF.Gelu, bias=cb1[:, 0:1])
                ops_ = pAG.tile([20, 64], f32, tag="ag", name="ops")
                mm(ops_[:, 0:jn * 4], lhsT=cw2[:], rhs=hg[:, 0:jn * 4],
                   start=True, stop=True)
                nc.scalar.activation(outsb[:, bq * 64:bq * 64 + jn * 4],
                                     ops_[:, 0:jn * 4], AF.Identity,
                                     bias=cb2[:, 0:1])
            nc.sync.dma_start(d_out[:], outsb[:])

    nc.finalize()
    return nc


def _assign_groups(ne, n_groups):
    """LPT bin packing: samples -> groups of <=GRP samples, balancing edge
    counts. Returns (group_of_sample, slot_of_sample, max_load)."""
    import heapq
    B = ne.shape[0]
    order = np.argsort(-ne, kind="stable")
    loads = [0] * n_groups
    counts = [0] * n_groups
    gof = np.zeros(B, np.int64)
    sof = np.zeros(B, np.int64)
    hp = [(0, g) for g in range(n_groups)]
    heapq.heapify(hp)
    for s in order:
        while True:
            ld, g = heapq.heappop(hp)
            if counts[g] < GRP and ld == loads[g]:
                break
        gof[s] = g
        sof[s] = counts[g]
        counts[g] += 1
        loads[g] += int(ne[s])
        if counts[g] < GRP:
            heapq.heappush(hp, (loads[g], g))
    return gof, sof, max(loads)


def _host_prep(inputs, G=None):
    f, hh = np.float32, np.float16
    src = np.asarray(inputs["edge_src"], np.int64)
    rel = np.asarray(inputs["edge_rel"], np.int64)
    tgt = np.asarray(inputs["edge_tgt"], np.int64)
    ne = np.asarray(inputs["n_edges"], np.int64)
    qs = np.asarray(inputs["query_src"], np.int64)
    qt = np.asarray(inputs["query_tgt"], np.int64)
    B = src.shape[0]
    if G is None:
        bc = -(-B // N_CORES)
        G = max(28, (-(-bc // GRP) + 3) // 4 * 4)
    while True:
        gof, sof, maxload = _assign_groups(ne, G * N_CORES)
        if maxload <= EC:
            break
        G += 4
    NG = G * N_CORES
    NQ = G // 4

    oh = np.zeros((NG, P, 3 * EC), hh)
    relt = np.zeros((NG // 4, P, EC), hh)
    indt = np.zeros((NG // 4, P, EC), hh)
    qoh = np.zeros((NG, P, 8), hh)
    ecnt = np.zeros(NG, np.int64)
    for s in range(B):
        g = int(gof[s])
        so = int(sof[s]) * N_ENT
        k = int(ne[s])
        q4, b4 = divmod(g, 4)
        if k:
            e0 = int(ecnt[g])
            ecnt[g] += k
            idx = np.arange(e0, e0 + k)
            es, et, er = src[s, :k], tgt[s, :k], rel[s, :k]
            oh[g, so + es, idx] = 1
            oh[g, so + et, EC + idx] = 1
            oh[g, idx, 2 * EC + so + et] = 1
            relt[q4, b4 * 32 + er, idx] = 1
            np.add.at(indt, (q4, b4 * 32, so + et), np.float16(1.0))
        qoh[g, so + qs[s], sof[s]] = 1
        qoh[g, so + qt[s], 4 + sof[s]] = 1

    # shared weights
    ee = np.asarray(inputs["entity_embed"], f)
    W1 = np.asarray(inputs["msg_W1"], f)
    reltab = (np.asarray(inputs["rel_embed"], f) @ W1[128:256]
              + np.asarray(inputs["msg_b1"], f))
    rt4 = np.zeros((P, 256), f)
    b2row = np.zeros((P, P), f)
    for rb in (0, 32, 64, 96):
        rt4[rb:rb + 20] = reltab
        b2row[rb] = np.asarray(inputs["msg_b2"], f)
    w2m_ = np.asarray(inputs["msg_W2"], f)
    w1u_ = np.asarray(inputs["upd_W1"], f)
    w2u_ = np.asarray(inputs["upd_W2"], f)
    cw1_ = np.asarray(inputs["cls_W1"], f)

    shared = {
        "s0": np.tile(ee.T, (1, GRP * G)).astype(hh),
        "w1s": W1[0:128].astype(hh).copy(),
        "w1t": W1[256:384].astype(hh).copy(),
        "rt4": rt4.astype(hh),
        "w2m": np.concatenate([w2m_[0:128], w2m_[128:256]], axis=1).astype(hh),
        "w1u": np.concatenate(
            [w1u_[0:128, 0:128], w1u_[0:128, 128:256],
             w1u_[128:256, 0:128], w1u_[128:256, 128:256]], axis=1).astype(hh),
        "w2u": np.concatenate([w2u_[0:128], w2u_[128:256]], axis=1).astype(hh),
        "b2row": b2row.astype(hh),
        "b1u": np.asarray(inputs["upd_b1"], f).reshape(2, 128).T.copy(),
        "b2u": np.asarray(inputs["upd_b2"], f).reshape(128, 1).copy(),
        "cw1": np.concatenate([cw1_[0:128], cw1_[128:256]], axis=1).astype(f),
        "cb1": np.asarray(inputs["cls_b1"], f).reshape(128, 1).copy(),
        "cw2": np.asarray(inputs["cls_W2"], f).copy(),
        "cb2": np.asarray(inputs["cls_b2"], f).reshape(20, 1).copy(),
    }

    in_maps = []
    for c in range(N_CORES):
        gsl = slice(c * G, (c + 1) * G)
        qsl = slice(c * NQ, (c + 1) * NQ)
        m = dict(shared)
        m["oh"] = np.ascontiguousarray(oh[gsl])
        m["relt"] = np.ascontiguousarray(relt[qsl])
        m["indt"] = np.ascontiguousarray(indt[qsl])
        m["qoh"] = np.ascontiguousarray(
            qoh[gsl].transpose(1, 0, 2).reshape(P, G * 8))
        in_maps.append(m)
    return in_maps, gof, sof, G


_CACHE = {}


def kernel(**inputs):
    B = np.asarray(inputs["edge_src"]).shape[0]
    in_maps, gof, sof, G = _host_prep(inputs)

    key = G
    if key not in _CACHE:
        _CACHE[key] = _build_nc(G, N_STEPS)
    nc = _CACHE[key]

    from concourse.bass_utils import run_bass_kernel_spmd
    res = run_bass_kernel_spmd(nc, in_maps, core_ids=list(range(N_CORES)))

    out = np.empty((B, N_REL), np.float32)
    allc = np.concatenate([r["out"].T for r in res.results], axis=0)
    # row index in allc: core * (G*4) + (g_local*4 + slot) = gof*4 + sof
    out[:, :] = allc[gof * 4 + sof]
    return np.ascontiguousarray(out)
